# revision 9
# baseline (speedup 1.0000x reference)
"""Bass/Trainium2 kernel for the BarlowTwins-style cross-entropy loss.

Reference (per batch b of 8):
    logits = z1[b].T @ z2[b] / T            (2048 x 2048, K=256, T=1.0)
    logp   = log_softmax(logits, axis=0)    (softmax over first axis n)
    loss   = -mean_b,m logp[m, m]

Sharding: pure data parallel over the batch axis b -> one batch element per
NeuronCore (8 cores).  Each core computes logitsT[m, n] = sum_s z2[s,m]*z1[s,n]
so the softmax reduction runs along the free axis.  Per 128-row chunk of m:
    mx[m]  = -max_n logitsT[m, n]           (negated row max, DVE reduce)
    se[m]  = sum_n exp(logitsT[m, n] + mx[m]) (ACT exp+accumulate, split in two
                                             halves so PSUM banks release early)
    dgblk  = logitsT[:, diag block]         (raw [128,128] block copied to SBUF
                                             on ACT/DVE alternately, DMA'd out;
                                             host gathers the diagonal)
The host combines: loss = -mean(dg + mx - log(se)).

Inputs are converted to bf16 on the host (halves DMA traffic; PE runs bf16 at
full rate with f32 PSUM accumulation; loss error vs f32 reference ~1e-5).
"""

import numpy as np
import ml_dtypes

import concourse.bass as bass
import concourse.tile as tile
from concourse import bacc, mybir
from concourse.bass_utils import run_bass_kernel_spmd

B = 8          # batch (one element per core)
S = 256        # contraction dim
N = 2048       # feature dim (n and m)
P = 128        # SBUF partitions
KC = S // P    # 2 contraction chunks
MC = N // P    # 16 row chunks of logitsT
NB = N // 512  # 4 moving-dim blocks per matmul row chunk
TEMPERATURE = 1.0

_CACHE = {}


def _build():
    if "nc" in _CACHE:
        return _CACHE["nc"]

    f32 = mybir.dt.float32
    bf16 = mybir.dt.bfloat16

    nc = bacc.Bacc("TRN2", target_bir_lowering=False, debug=False)
    z1 = nc.dram_tensor("z1", [S, N], bf16, kind="ExternalInput").ap()
    z2 = nc.dram_tensor("z2", [S, N], bf16, kind="ExternalInput").ap()
    se_d = nc.dram_tensor("se", [P, 2 * MC], f32, kind="ExternalOutput").ap()
    mx_d = nc.dram_tensor("mx", [P, MC], f32, kind="ExternalOutput").ap()
    dg_d = nc.dram_tensor("dgblk", [MC, P, P], f32, kind="ExternalOutput").ap()

    z1r = z1.rearrange("(k p) n -> k p n", p=P)
    z2r = z2.rearrange("(k p) n -> k p n", p=P)

    with tile.TileContext(nc) as tc:
        with (
            tc.tile_pool(name="const", bufs=1) as cpool,
            tc.tile_pool(name="zb", bufs=1) as zpool,
            tc.tile_pool(name="psum", bufs=2, space="PSUM") as ppool,
            tc.tile_pool(name="expout", bufs=4) as epool,
            tc.tile_pool(name="dscr", bufs=3) as dpool,
            tc.tile_pool(name="mx", bufs=4) as mpool,
        ):
            # ACT exp-table preload, overlapped with the input DMAs.
            dummy = cpool.tile([1, 1], f32, tag="dummy")
            nc.gpsimd.memset(dummy[:], 0.0)
            nc.scalar.activation(
                dummy[:], dummy[:], mybir.ActivationFunctionType.Exp, bias=0.0
            )

            se_sb = cpool.tile([P, 2 * MC], f32, tag="se_sb")
            mx_sb = cpool.tile([P, MC], f32, tag="mx_sb")

            # Input loads: lead with the slices the first row chunk needs so
            # the matmuls start early, then stream the rest.
            z1b = [
                zpool.tile([P, N], bf16, name=f"z1b{k}", tag=f"z1b{k}")
                for k in range(KC)
            ]
            z2b = [
                zpool.tile([P, N], bf16, name=f"z2b{k}", tag=f"z2b{k}")
                for k in range(KC)
            ]
            for k in range(KC):
                nc.sync.dma_start(z2b[k][:, 0:128], z2r[k][:, 0:128])
            for k in range(KC):
                nc.sync.dma_start(z1b[k][:, 0:512], z1r[k][:, 0:512])
            for k in range(KC):
                nc.sync.dma_start(z1b[k][:, 512:N], z1r[k][:, 512:N])
            for k in range(KC):
                nc.sync.dma_start(z2b[k][:, 128:1024], z2r[k][:, 128:1024])
            for k in range(KC):
                nc.sync.dma_start(z2b[k][:, 1024:N], z2r[k][:, 1024:N])

            H = N // 2
            inv_t = 1.0 / TEMPERATURE
            for m in range(MC):
                ms = slice(m * P, (m + 1) * P)
                psum = ppool.tile([P, N], f32, tag="psum")
                for k in range(KC):
                    for nb in range(NB):
                        ns = slice(nb * 512, (nb + 1) * 512)
                        nc.tensor.matmul(
                            psum[:, ns],
                            lhsT=z2b[k][:, ms],
                            rhs=z1b[k][:, ns],
                            start=(k == 0),
                            stop=(k == KC - 1),
                        )

                # raw diagonal block -> SBUF -> DRAM (host gathers diagonal);
                # alternate the copy engine to balance ACT and DVE load
                dscr = dpool.tile([P, P], f32, tag="dscr")
                if m % 2 == 0:
                    nc.scalar.copy(dscr[:], psum[:, ms])
                else:
                    nc.vector.tensor_copy(dscr[:], psum[:, ms])
                nc.sync.dma_start(dg_d[m], dscr[:])

                # negated row max of the [P, N] chunk
                mx_t = mpool.tile([P, 1], f32, tag="mx")
                nc.vector.tensor_reduce(
                    mx_t[:],
                    psum[:],
                    axis=mybir.AxisListType.X,
                    op=mybir.AluOpType.max,
                    negate=True,
                )
                nc.vector.tensor_copy(mx_sb[:, m : m + 1], mx_t[:])

                # exp(logitsT - rowmax) accumulated along the row, in two
                # halves so PSUM banks free up for the next-next chunk early
                for h in range(2):
                    hs = slice(h * H, (h + 1) * H)
                    eo = epool.tile([P, H], bf16, tag="eo")
                    nc.scalar.activation(
                        eo[:],
                        psum[:, hs],
                        mybir.ActivationFunctionType.Exp,
                        bias=mx_t[:],
                        scale=inv_t,
                        accum_out=se_sb[:, 2 * m + h : 2 * m + h + 1],
                    )

            nc.sync.dma_start(se_d[:], se_sb[:])
            nc.sync.dma_start(mx_d[:], mx_sb[:])

    nc.compile()
    _CACHE["nc"] = nc
    return nc


def _run(z1, z2, **spmd_kwargs):
    """Shard over batch, run on 8 cores, return (loss, BassKernelResults)."""
    nc = _build()
    z1 = np.ascontiguousarray(z1)
    z2 = np.ascontiguousarray(z2)
    in_maps = [
        {
            "z1": np.ascontiguousarray(z1[b].astype(ml_dtypes.bfloat16)),
            "z2": np.ascontiguousarray(z2[b].astype(ml_dtypes.bfloat16)),
        }
        for b in range(B)
    ]
    res = run_bass_kernel_spmd(nc, in_maps, core_ids=list(range(B)), **spmd_kwargs)

    total = 0.0
    pidx = np.arange(P)
    for b in range(B):
        se2 = res.results[b]["se"].astype(np.float64)  # [P, 2*MC] half-sums
        mx = res.results[b]["mx"].astype(np.float64)   # [P, MC] negated rowmax
        # dgblk[mc, p, :] holds logitsT[mc*128+p, mc*128 : (mc+1)*128];
        # the diagonal element for row m = mc*128+p sits at column p.
        dg = res.results[b]["dgblk"][:, pidx, pidx].astype(np.float64)  # [MC, P]
        se = se2[:, 0::2] + se2[:, 1::2]               # [P, MC]
        logZ = -mx + np.log(se)                        # [P, MC]
        total += np.sum(dg.T - logZ)
    loss = -total / (B * N)
    return np.asarray(loss, dtype=np.float32), res


def kernel(z1, z2):
    loss, _ = _run(z1, z2)
    return loss


# revision 10
# speedup vs baseline: 1.3718x; 1.3718x over previous
"""Bass/Trainium2 kernel for the BarlowTwins-style cross-entropy loss.

Reference (per batch b of 8):
    logits = z1[b].T @ z2[b] / T            (2048 x 2048, K=256, T=1.0)
    logp   = log_softmax(logits, axis=0)    (softmax over first axis n)
    loss   = -mean_b,m logp[m, m]

Sharding: pure data parallel over the batch axis b -> one batch element per
NeuronCore (8 cores).  Each core computes logitsT[m, n] = sum_s z2[s,m]*z1[s,n]
so the softmax reduction runs along the free axis.  Online-softmax style, each
128-row chunk of m is processed as two independent 1024-column halves L/R so
the max -> exp chain of one half overlaps the matmuls of the other:
    nmx_h[m] = -max_{n in h} logitsT[m, n]     (DVE reduce, per half)
    se_h[m]  = sum_{n in h} exp(logitsT[m, n] + nmx_h[m])  (ACT exp+accum)
    dgblk    = raw [128,128] block with the diagonal (DVE copy -> DMA; host
               gathers the diagonal)
Host merge: M = max(-nmx_a, -nmx_b); se = se_a*e^(-nmx_a-M) + se_b*e^(-nmx_b-M)
            loss = -mean(dg - M - log(se)).

Inputs are converted to bf16 on the host (halves DMA traffic; PE runs bf16 at
full rate with f32 PSUM accumulation; loss error vs f32 reference ~1e-5).
"""

import numpy as np
import ml_dtypes

import concourse.bass as bass
import concourse.tile as tile
from concourse import bacc, mybir
from concourse.bass_utils import run_bass_kernel_spmd

B = 8          # batch (one element per core)
S = 256        # contraction dim
N = 2048       # feature dim (n and m)
P = 128        # SBUF partitions
KC = S // P    # 2 contraction chunks
MC = N // P    # 16 row chunks of logitsT
H = N // 2     # half width for the online-softmax split
TEMPERATURE = 1.0

_CACHE = {}


def _build():
    if "nc" in _CACHE:
        return _CACHE["nc"]

    f32 = mybir.dt.float32
    bf16 = mybir.dt.bfloat16

    nc = bacc.Bacc("TRN2", target_bir_lowering=False, debug=False)
    z1 = nc.dram_tensor("z1", [S, N], bf16, kind="ExternalInput").ap()
    z2 = nc.dram_tensor("z2", [S, N], bf16, kind="ExternalInput").ap()
    se_d = nc.dram_tensor("se", [P, 2 * MC], f32, kind="ExternalOutput").ap()
    mx_d = nc.dram_tensor("mx", [P, 2 * MC], f32, kind="ExternalOutput").ap()
    dg_d = nc.dram_tensor("dgblk", [MC, P, P], f32, kind="ExternalOutput").ap()

    z1r = z1.rearrange("(k p) n -> k p n", p=P)
    z2r = z2.rearrange("(k p) n -> k p n", p=P)

    with tile.TileContext(nc) as tc:
        with (
            tc.tile_pool(name="const", bufs=1) as cpool,
            tc.tile_pool(name="zb", bufs=1) as zpool,
            tc.tile_pool(name="psum", bufs=4, space="PSUM") as ppool,
            tc.tile_pool(name="expout", bufs=4) as epool,
            tc.tile_pool(name="dscr", bufs=3) as dpool,
            tc.tile_pool(name="mx", bufs=6) as mpool,
        ):
            # ACT exp-table preload, overlapped with the input DMAs.
            dummy = cpool.tile([1, 1], f32, tag="dummy")
            nc.gpsimd.memset(dummy[:], 0.0)
            nc.scalar.activation(
                dummy[:], dummy[:], mybir.ActivationFunctionType.Exp, bias=0.0
            )

            se_sb = cpool.tile([P, 2 * MC], f32, tag="se_sb")
            mx_sb = cpool.tile([P, 2 * MC], f32, tag="mx_sb")

            # Input loads: lead with the slices the first row chunk's left
            # half needs so the matmuls start early, then stream the rest.
            z1b = [
                zpool.tile([P, N], bf16, name=f"z1b{k}", tag=f"z1b{k}")
                for k in range(KC)
            ]
            z2b = [
                zpool.tile([P, N], bf16, name=f"z2b{k}", tag=f"z2b{k}")
                for k in range(KC)
            ]
            for k in range(KC):
                nc.sync.dma_start(z2b[k][:, 0:128], z2r[k][:, 0:128])
            for k in range(KC):
                nc.sync.dma_start(z1b[k][:, 0:H], z1r[k][:, 0:H])
            for k in range(KC):
                nc.sync.dma_start(z1b[k][:, H:N], z1r[k][:, H:N])
            for k in range(KC):
                nc.sync.dma_start(z2b[k][:, 128:1024], z2r[k][:, 128:1024])
            for k in range(KC):
                nc.sync.dma_start(z2b[k][:, 1024:N], z2r[k][:, 1024:N])

            inv_t = 1.0 / TEMPERATURE
            for m in range(MC):
                ms = slice(m * P, (m + 1) * P)
                for h in range(2):  # left / right half of the row chunk
                    hbase = h * H
                    psum = ppool.tile([P, H], f32, tag="psum")
                    for k in range(KC):
                        for nb in range(2):
                            ns = slice(hbase + nb * 512, hbase + (nb + 1) * 512)
                            nc.tensor.matmul(
                                psum[:, nb * 512 : (nb + 1) * 512],
                                lhsT=z2b[k][:, ms],
                                rhs=z1b[k][:, ns],
                                start=(k == 0),
                                stop=(k == KC - 1),
                            )

                    # negated half-row max
                    mx_t = mpool.tile([P, 1], f32, tag="mx")
                    nc.vector.tensor_reduce(
                        mx_t[:],
                        psum[:],
                        axis=mybir.AxisListType.X,
                        op=mybir.AluOpType.max,
                        negate=True,
                    )
                    nc.vector.tensor_copy(mx_sb[:, 2 * m + h : 2 * m + h + 1], mx_t[:])

                    # the diagonal block lives in one specific half
                    if m * P >= hbase and m * P < hbase + H:
                        dscr = dpool.tile([P, P], f32, tag="dscr")
                        nc.vector.tensor_copy(dscr[:], psum[:, m * P - hbase : m * P - hbase + P])
                        nc.sync.dma_start(dg_d[m], dscr[:])

                    # exp(logitsT - halfmax), accumulated along the half row
                    eo = epool.tile([P, H], bf16, tag="eo")
                    nc.scalar.activation(
                        eo[:],
                        psum[:],
                        mybir.ActivationFunctionType.Exp,
                        bias=mx_t[:],
                        scale=inv_t,
                        accum_out=se_sb[:, 2 * m + h : 2 * m + h + 1],
                    )

            nc.sync.dma_start(se_d[:], se_sb[:])
            nc.sync.dma_start(mx_d[:], mx_sb[:])

    nc.compile()
    _CACHE["nc"] = nc
    return nc


def _run(z1, z2, **spmd_kwargs):
    """Shard over batch, run on 8 cores, return (loss, BassKernelResults)."""
    nc = _build()
    z1 = np.ascontiguousarray(z1)
    z2 = np.ascontiguousarray(z2)
    in_maps = [
        {
            "z1": np.ascontiguousarray(z1[b].astype(ml_dtypes.bfloat16)),
            "z2": np.ascontiguousarray(z2[b].astype(ml_dtypes.bfloat16)),
        }
        for b in range(B)
    ]
    res = run_bass_kernel_spmd(nc, in_maps, core_ids=list(range(B)), **spmd_kwargs)

    total = 0.0
    pidx = np.arange(P)
    for b in range(B):
        se2 = res.results[b]["se"].astype(np.float64)  # [P, 2*MC] half sums
        nmx = res.results[b]["mx"].astype(np.float64)  # [P, 2*MC] negated half maxes
        dg = res.results[b]["dgblk"][:, pidx, pidx].astype(np.float64)  # [MC, P]
        ma = -nmx[:, 0::2]                             # [P, MC] left-half max
        mb = -nmx[:, 1::2]
        sea = se2[:, 0::2]
        seb = se2[:, 1::2]
        M = np.maximum(ma, mb)
        se = sea * np.exp(ma - M) + seb * np.exp(mb - M)
        logZ = M + np.log(se)                          # [P, MC]
        total += np.sum(dg.T - logZ)
    loss = -total / (B * N)
    return np.asarray(loss, dtype=np.float32), res


def kernel(z1, z2):
    loss, _ = _run(z1, z2)
    return loss
